# revision 28
# baseline (speedup 1.0000x reference)
"""Trainium2 kernel for nn_CustomSTFT (STFT -> mag/phase -> iSTFT round-trip).

Math: the reference's magnitude/phase decomposition followed by
re-composition (rr = mag*cos(phase), ri = mag*sin(phase)) is the exact
identity rr = sr, ri = si.  Therefore

    contrib = fr @ (DFT_R @ IDFT_R.T - DFT_I @ IDFT_I.T)

and the matrix M = DFT_R @ IDFT_R.T - DFT_I @ IDFT_I.T is exactly diagonal
(DFT orthogonality; verified to 3e-14 in float64), i.e. contrib[b,t,:] =
fr[b,t,:] * d with d = diag(M).  Overlap-add of x_pad-framed windows times d
collapses to a pointwise scale: out[b, j] = x_pad[b, j] * S(j), where
S(j) = sum_t d[j - HOP*t].  After center-cropping, the output region only
touches x_pad[:, PAD:-PAD] == input, so

    out[b, :] = input[b, :] * S_mid,   S_mid = S[PAD:-PAD]  (precomputable)

S_mid is periodic with period HOP=200 except the first/last ~200 samples.
The device kernel is thus a DMA-roofline elementwise multiply, data-parallel
over 8 cores (batch sharded, 4 rows/core), each core viewing its [4, 480000]
shard as [128, 15000] (contiguous reshape) and streaming column chunks.
"""

from contextlib import ExitStack

import numpy as np

import concourse.bass as bass
import concourse.mybir as mybir
from concourse.bass_utils import run_bass_kernel_spmd

N_FFT = 800
HOP = 200
FREQ = N_FFT // 2 + 1  # 401
PAD = N_FFT // 2       # 400

N_CORES = 8
B, T = 32, 480000
BPC = B // N_CORES          # 4 batch rows per core
ROWS, COLS = 128, 15000     # per-core view: 4*480000 == 128*15000
W = 1000                    # chunk width; multiple of HOP so scale phase is 0
NCHUNK = COLS // W          # 15 chunks: fine-grained load/mul/store pipelining
EDGE = 400                  # edge-correction width (deviation is <= 200)


def _host_scales(w=W):
    """Scale chunk [128, w] (periodic) + edge rows, in float64."""
    n = np.arange(N_FFT, dtype=np.float64)
    angle = 2.0 * np.pi * np.outer(n[:FREQ], n) / N_FFT
    win = np.hanning(N_FFT)
    dft_r = (np.cos(angle) * win).T
    dft_i = (-np.sin(angle) * win).T
    idft_r = np.cos(angle).T / FREQ
    idft_i = np.sin(angle).T / FREQ
    for m in (idft_r, idft_i):
        m[:, 0] *= 0.5
        m[:, -1] *= 0.5
    # diagonal of DFT_R @ IDFT_R.T - DFT_I @ IDFT_I.T (off-diagonal is 0)
    d = np.einsum("wf,wf->w", dft_r, idft_r) - np.einsum("wf,wf->w", dft_i, idft_i)

    T_pad = T + 2 * PAD
    frames = (T_pad - N_FFT) // HOP + 1
    S = np.zeros(T_pad, dtype=np.float64)
    for t in range(frames):
        S[t * HOP : t * HOP + N_FFT] += d
    S_mid = S[PAD:-PAD]  # [T]

    period = S_mid[PAD : PAD + HOP]  # interior period (any aligned offset >= 200)
    # One combined constant tensor [128, W + 2*EDGE]:
    #   cols [0, W)            periodic mid scale (identical rows)
    #   cols [W, W+EDGE)       scale for the first EDGE samples of n==0 rows
    #   cols [W+EDGE, W+2E)    scale for the last EDGE samples of n==31 rows
    # Partition p = b*32 + n covers time [n*15000, (n+1)*15000); only n==0
    # rows (p%32==0) deviate at the start and n==31 rows (p%32==31) at the
    # end — other rows get the periodic values in the edge columns too.
    const = np.empty((ROWS, w + 2 * EDGE), dtype=np.float32)
    const[:, :w] = np.tile(period, w // HOP).astype(np.float32)
    const[:, w:] = np.tile(period, 2 * EDGE // HOP).astype(np.float32)
    for p in range(0, ROWS, 32):
        const[p, w : w + EDGE] = S_mid[:EDGE].astype(np.float32)
    for p in range(31, ROWS, 32):
        const[p, w + EDGE :] = S_mid[-EDGE:].astype(np.float32)
    return const


def _build_nc(repeat=1, w=W):
    # Raw Bass (no Tile): this toolchain's codegen accepts at most one
    # embedded sem wait per instruction, so all waits are standalone
    # wait_ge sequencer commands and DMA completions tick explicit sems.
    # `repeat` re-runs the whole body K times inside one NEFF (benchmarking
    # only — isolates device time from dispatch overhead via the slope).
    nchunk = COLS // w
    assert COLS % w == 0 and w % HOP == 0
    nc = bass.Bass()
    x = nc.declare_dram_parameter("x", [ROWS, COLS], mybir.dt.float32, isOutput=False)
    sc = nc.declare_dram_parameter(
        "scale", [ROWS, w + 2 * EDGE], mybir.dt.float32, isOutput=False
    )
    y = nc.declare_dram_parameter("y", [ROWS, COLS], mybir.dt.float32, isOutput=True)

    f32 = mybir.dt.float32
    with ExitStack() as ctx:
        sct = ctx.enter_context(nc.sbuf_tensor("sct", [ROWS, w + 2 * EDGE], f32))
        its = [
            ctx.enter_context(nc.sbuf_tensor(f"it{c}", [ROWS, w], f32))
            for c in range(nchunk)
        ]
        ots = [
            ctx.enter_context(nc.sbuf_tensor(f"ot{c}", [ROWS, w], f32))
            for c in range(nchunk)
        ]
        s_const = ctx.enter_context(nc.semaphore("s_const"))
        s_in = [ctx.enter_context(nc.semaphore(f"s_in{c}")) for c in range(nchunk)]
        s_mul = [ctx.enter_context(nc.semaphore(f"s_mul{c}")) for c in range(nchunk)]
        s_outc = [ctx.enter_context(nc.semaphore(f"s_outc{c}")) for c in range(nchunk)]
        block = ctx.enter_context(nc.Block())

        # Per-chunk multiply segments (col_lo, col_hi, scale_off): scale for
        # cols [lo, hi) is sct[:, off : off + hi - lo]. Chunk starts are
        # multiples of HOP, so the periodic mid scale is phase-aligned for
        # any in-chunk offset.
        dve_muls = []
        for c in range(nchunk):
            segs = []
            lo, hi = 0, w
            if c == 0:
                segs.append((0, EDGE, w))  # row-dependent start-edge scale
                lo = EDGE
            if c == nchunk - 1:
                hi = w - EDGE
            if hi > lo:
                segs.append((lo, hi, lo))
            if c == nchunk - 1:
                segs.append((w - EDGE, w, w + EDGE))
            dve_muls.append(segs)
        n_muls = [len(dve_muls[c]) for c in range(nchunk)]

        # Input stream on the SP HW-DGE ring, output stream on the ACT
        # HW-DGE ring: HWDGE DMAs execute FIFO per issuing engine, so
        # splitting directions across the two physical rings lets loads
        # and stores overlap instead of serializing.
        @block.sync
        def _(sync):
            sync.dma_start(out=sct[:], in_=sc[:]).then_inc(s_const, 16)
            for k in range(repeat):
                for c in range(nchunk):
                    if k > 0:  # its[c] free once iteration k-1's muls consumed it
                        sync.wait_ge(s_mul[c], n_muls[c] * k)
                    sync.dma_start(
                        out=its[c][:], in_=x[:, c * w : (c + 1) * w]
                    ).then_inc(s_in[c], 16)

        @block.scalar
        def _(scalar):
            for k in range(repeat):
                for c in range(nchunk):
                    scalar.wait_ge(s_mul[c], n_muls[c] * (k + 1))
                    scalar.dma_start(
                        out=y[:, c * w : (c + 1) * w], in_=ots[c][:]
                    ).then_inc(s_outc[c], 16)
            for c in range(nchunk):
                scalar.wait_ge(s_outc[c], 16 * repeat)

        @block.vector
        def _(vector):
            vector.wait_ge(s_const, 16)
            for k in range(repeat):
                for c in range(nchunk):
                    vector.wait_ge(s_in[c], 16 * (k + 1))
                    if k > 0:  # ots[c] free once iteration k-1's store drained
                        vector.wait_ge(s_outc[c], 16 * k)
                    it, ot = its[c], ots[c]
                    for lo, hi, off in dve_muls[c]:
                        nc.vector.tensor_mul(
                            ot[:, lo:hi], it[:, lo:hi], sct[:, off : off + hi - lo]
                        ).then_inc(s_mul[c], 1)
    return nc


_CACHE = {}


def _get_compiled():
    if "nc" not in _CACHE:
        _CACHE["nc"] = _build_nc()
        _CACHE["scales"] = _host_scales()
    return _CACHE["nc"], _CACHE["scales"]


def kernel(input_data):
    x = np.ascontiguousarray(np.asarray(input_data, dtype=np.float32))
    assert x.shape == (B, T), x.shape
    nc, const = _get_compiled()
    in_maps = [
        {
            "x": x[c * BPC : (c + 1) * BPC].reshape(ROWS, COLS),
            "scale": const,
        }
        for c in range(N_CORES)
    ]
    res = run_bass_kernel_spmd(nc, in_maps, core_ids=list(range(N_CORES)))
    out = np.concatenate(
        [res.results[c]["y"].reshape(BPC, T) for c in range(N_CORES)], axis=0
    )
    return out.reshape(B, 1, T)


# revision 31
# speedup vs baseline: 1.0601x; 1.0601x over previous
"""Trainium2 kernel for nn_CustomSTFT (STFT -> mag/phase -> iSTFT round-trip).

Math: the reference's magnitude/phase decomposition followed by
re-composition (rr = mag*cos(phase), ri = mag*sin(phase)) is the exact
identity rr = sr, ri = si.  Therefore

    contrib = fr @ (DFT_R @ IDFT_R.T - DFT_I @ IDFT_I.T)

and the matrix M = DFT_R @ IDFT_R.T - DFT_I @ IDFT_I.T is exactly diagonal
(DFT orthogonality; verified to 3e-14 in float64), i.e. contrib[b,t,:] =
fr[b,t,:] * d with d = diag(M).  Overlap-add of x_pad-framed windows times d
collapses to a pointwise scale: out[b, j] = x_pad[b, j] * S(j), where
S(j) = sum_t d[j - HOP*t].  After center-cropping, the output region only
touches x_pad[:, PAD:-PAD] == input, so

    out[b, :] = input[b, :] * S_mid,   S_mid = S[PAD:-PAD]  (precomputable)

S_mid is periodic with period HOP=200 except the first/last ~200 samples.
The device kernel is thus a DMA-roofline elementwise multiply, data-parallel
over 8 cores (batch sharded, 4 rows/core), each core viewing its [4, 480000]
shard as [128, 15000] (contiguous reshape) and streaming column chunks.
"""

from contextlib import ExitStack

import numpy as np

import concourse.bass as bass
import concourse.mybir as mybir
from concourse.bass_utils import run_bass_kernel_spmd

N_FFT = 800
HOP = 200
FREQ = N_FFT // 2 + 1  # 401
PAD = N_FFT // 2       # 400

N_CORES = 8
B, T = 32, 480000
BPC = B // N_CORES          # 4 batch rows per core
ROWS, COLS = 128, 15000     # per-core view: 4*480000 == 128*15000
W = 1000                    # chunk width; multiple of HOP so scale phase is 0
NCHUNK = COLS // W          # 15 chunks: fine-grained load/mul/store pipelining
EDGE = 400                  # edge-correction width (deviation is <= 200)


def _host_scales(w=W):
    """Scale chunk [128, w] (periodic) + edge rows, in float64."""
    n = np.arange(N_FFT, dtype=np.float64)
    angle = 2.0 * np.pi * np.outer(n[:FREQ], n) / N_FFT
    win = np.hanning(N_FFT)
    dft_r = (np.cos(angle) * win).T
    dft_i = (-np.sin(angle) * win).T
    idft_r = np.cos(angle).T / FREQ
    idft_i = np.sin(angle).T / FREQ
    for m in (idft_r, idft_i):
        m[:, 0] *= 0.5
        m[:, -1] *= 0.5
    # diagonal of DFT_R @ IDFT_R.T - DFT_I @ IDFT_I.T (off-diagonal is 0)
    d = np.einsum("wf,wf->w", dft_r, idft_r) - np.einsum("wf,wf->w", dft_i, idft_i)

    T_pad = T + 2 * PAD
    frames = (T_pad - N_FFT) // HOP + 1
    S = np.zeros(T_pad, dtype=np.float64)
    for t in range(frames):
        S[t * HOP : t * HOP + N_FFT] += d
    S_mid = S[PAD:-PAD]  # [T]

    period = S_mid[PAD : PAD + HOP]  # interior period (any aligned offset >= 200)
    # One combined constant tensor [128, HOP + 2*EDGE] (the periodic mid
    # scale is read via a step-0 broadcast AP, so one period suffices):
    #   cols [0, HOP)               one period of the mid scale
    #   cols [HOP, HOP+EDGE)        scale for the first EDGE samples, n==0 rows
    #   cols [HOP+EDGE, HOP+2E)     scale for the last EDGE samples, n==31 rows
    # Partition p = b*32 + n covers time [n*15000, (n+1)*15000); only n==0
    # rows (p%32==0) deviate at the start and n==31 rows (p%32==31) at the
    # end — other rows get the periodic values in the edge columns too.
    const = np.empty((ROWS, HOP + 2 * EDGE), dtype=np.float32)
    const[:, :HOP] = period.astype(np.float32)
    const[:, HOP:] = np.tile(period, 2 * EDGE // HOP).astype(np.float32)
    for p in range(0, ROWS, 32):
        const[p, HOP : HOP + EDGE] = S_mid[:EDGE].astype(np.float32)
    for p in range(31, ROWS, 32):
        const[p, HOP + EDGE :] = S_mid[-EDGE:].astype(np.float32)
    return const


def _build_nc(repeat=1, w=W, gp_in=True, n_swq=4):
    # Raw Bass (no Tile): this toolchain's codegen accepts at most one
    # embedded sem wait per instruction, so all waits are standalone
    # wait_ge sequencer commands and DMA completions tick explicit sems.
    # `repeat` re-runs the whole body K times inside one NEFF (benchmarking
    # only — isolates device time from dispatch overhead via the slope).
    nchunk = COLS // w
    assert COLS % w == 0 and w % HOP == 0
    nc = bass.Bass(num_swdge_queues=n_swq)
    x = nc.declare_dram_parameter("x", [ROWS, COLS], mybir.dt.float32, isOutput=False)
    sc = nc.declare_dram_parameter(
        "scale", [ROWS, HOP + 2 * EDGE], mybir.dt.float32, isOutput=False
    )
    y = nc.declare_dram_parameter("y", [ROWS, COLS], mybir.dt.float32, isOutput=True)

    f32 = mybir.dt.float32
    with ExitStack() as ctx:
        sct = ctx.enter_context(nc.sbuf_tensor("sct", [ROWS, HOP + 2 * EDGE], f32))
        its = [
            ctx.enter_context(nc.sbuf_tensor(f"it{c}", [ROWS, w], f32))
            for c in range(nchunk)
        ]
        ots = [
            ctx.enter_context(nc.sbuf_tensor(f"ot{c}", [ROWS, w], f32))
            for c in range(nchunk)
        ]
        s_const = ctx.enter_context(nc.semaphore("s_const"))
        s_in = [ctx.enter_context(nc.semaphore(f"s_in{c}")) for c in range(nchunk)]
        s_mul = [ctx.enter_context(nc.semaphore(f"s_mul{c}")) for c in range(nchunk)]
        s_outc = [ctx.enter_context(nc.semaphore(f"s_outc{c}")) for c in range(nchunk)]
        block = ctx.enter_context(nc.Block())

        # Per-chunk multiply segments (col_lo, col_hi, scale_off, bcast).
        # Mid segments (bcast=True) read one period sct[:, :HOP] through a
        # step-0 broadcast AP; chunk starts and segment bounds are multiples
        # of HOP so the phase is always 0. Edge segments read their own
        # scale columns directly.
        dve_muls = []
        for c in range(nchunk):
            segs = []
            lo, hi = 0, w
            if c == 0:
                segs.append((0, EDGE, HOP, False))  # row-dependent start edge
                lo = EDGE
            if c == nchunk - 1:
                hi = w - EDGE
            if hi > lo:
                segs.append((lo, hi, 0, True))
            if c == nchunk - 1:
                segs.append((w - EDGE, w, HOP + EDGE, False))
            dve_muls.append(segs)
        n_muls = [len(dve_muls[c]) for c in range(nchunk)]

        # Input stream on the SP HW-DGE ring, output stream on the ACT
        # HW-DGE ring: HWDGE DMAs execute FIFO per issuing engine, so
        # splitting directions across the two physical rings lets loads
        # and stores overlap instead of serializing.
        @block.sync
        def _(sync):
            sync.dma_start(out=sct[:], in_=sc[:]).then_inc(s_const, 16)
            for k in range(repeat):
                for c in range(nchunk):
                    if gp_in and c % 2 == 1:
                        continue  # odd chunks loaded by the gpsimd SWDGE path
                    if k > 0:  # its[c] free once iteration k-1's muls consumed it
                        sync.wait_ge(s_mul[c], n_muls[c] * k)
                    sync.dma_start(
                        out=its[c][:], in_=x[:, c * w : (c + 1) * w]
                    ).then_inc(s_in[c], 16)

        if gp_in:

            @block.gpsimd
            def _(gpsimd):
                for k in range(repeat):
                    for c in range(1, nchunk, 2):
                        if k > 0:
                            gpsimd.wait_ge(s_mul[c], n_muls[c] * k)
                        gpsimd.dma_start(
                            out=its[c][:], in_=x[:, c * w : (c + 1) * w]
                        ).then_inc(s_in[c], 16)

        @block.scalar
        def _(scalar):
            for k in range(repeat):
                for c in range(nchunk):
                    scalar.wait_ge(s_mul[c], n_muls[c] * (k + 1))
                    scalar.dma_start(
                        out=y[:, c * w : (c + 1) * w], in_=ots[c][:]
                    ).then_inc(s_outc[c], 16)
            for c in range(nchunk):
                scalar.wait_ge(s_outc[c], 16 * repeat)

        @block.vector
        def _(vector):
            vector.wait_ge(s_const, 16)
            for k in range(repeat):
                for c in range(nchunk):
                    vector.wait_ge(s_in[c], 16 * (k + 1))
                    if k > 0:  # ots[c] free once iteration k-1's store drained
                        vector.wait_ge(s_outc[c], 16 * k)
                    it, ot = its[c], ots[c]
                    for lo, hi, off, bc in dve_muls[c]:
                        if bc:
                            nper = (hi - lo) // HOP
                            nc.vector.tensor_mul(
                                ot[:, lo:hi].rearrange("p (q t) -> p q t", t=HOP),
                                it[:, lo:hi].rearrange("p (q t) -> p q t", t=HOP),
                                sct[:, :HOP]
                                .rearrange("p (q t) -> p q t", q=1)
                                .broadcast_to([ROWS, nper, HOP]),
                            ).then_inc(s_mul[c], 1)
                        else:
                            nc.vector.tensor_mul(
                                ot[:, lo:hi], it[:, lo:hi], sct[:, off : off + hi - lo]
                            ).then_inc(s_mul[c], 1)
    return nc


_CACHE = {}


def _get_compiled():
    if "nc" not in _CACHE:
        _CACHE["nc"] = _build_nc()
        _CACHE["scales"] = _host_scales()
    return _CACHE["nc"], _CACHE["scales"]


def kernel(input_data):
    x = np.ascontiguousarray(np.asarray(input_data, dtype=np.float32))
    assert x.shape == (B, T), x.shape
    nc, const = _get_compiled()
    in_maps = [
        {
            "x": x[c * BPC : (c + 1) * BPC].reshape(ROWS, COLS),
            "scale": const,
        }
        for c in range(N_CORES)
    ]
    res = run_bass_kernel_spmd(nc, in_maps, core_ids=list(range(N_CORES)))
    out = np.concatenate(
        [res.results[c]["y"].reshape(BPC, T) for c in range(N_CORES)], axis=0
    )
    return out.reshape(B, 1, T)
